# revision 18
# baseline (speedup 1.0000x reference)
"""Additive (Bahdanau) attention on 8 TRN2 NeuronCores.

Reference computation (B=32, T=2048, H=1024):
    k = enc @ Wk.T + bk                  [B, T, H]
    q = dec @ Wq.T + bq                  [B, 1, H]
    s = tanh(k + q) @ Wout (+ bout)      [B, T]   (bout dropped: softmax shift-invariant)
    a = softmax(s, axis=-1)
    ctx = einsum('bt,bth->bh', a, enc)   -> [B, 1, H]

Sharding: data-parallel over B, 4 batches per core, no collectives.
Host prep: Wk pre-transposed; qb = dec@Wq.T + bq + bk precomputed (a per-batch
bias vector, 0.05% of the FLOPs); enc additionally shipped in [B, H, T] layout
so both contraction directions stream from HBM without on-chip transposes.

Per-core dataflow (per batch b, per T-tile of 512 rows), matmuls in fp32r:
    DMA  E^T [128p(h), 8i, 512t]   (single E stream; natural layout not needed)
    K^T psum[h'128, t512] += Wk'[h,h'] x E^T[h,t]      (contract h, 8 chunks)
    ACT: tanh(K^T + qb[h',b])  (per-partition bias, fused)
    score psum[1, t512] += Wout[h'] x tanhK^T          (8 chunk-matmuls, PE)
    ACT: p = exp(score), accum_out -> partial Z (fused row-sum)
    GpSimd: p_rep = broadcast p across partitions
    DVE: ctx_chunk[h] = accum_out of (E^T[h,t] * p_rep) -- fused mul+row-sum,
         one scalar_tensor_tensor per h-chunk; PE stays on the kt/score path
    end of batch: reduce tile partials, out[b] = PE-transpose(ctx * 1/Z)
"""

import numpy as np

B, T, H = 32, 2048, 1024
NCORES = 8
BPC = B // NCORES          # batches per core = 4
TT = 512                   # T-tile rows
NSUB = TT // 128           # 4 t-subtiles per tile
NTILES = T // TT           # 4 tiles per batch
HC = H // 128              # 8 chunks of the hidden dims
REPS = 1                   # benchmark-only: repeat main loop inside the NEFF


def _build():
    from concourse import bacc, mybir, tile, masks

    f32 = mybir.dt.float32
    f32r = mybir.dt.float32r
    AF = mybir.ActivationFunctionType

    nc = bacc.Bacc("TRN2", target_bir_lowering=False, debug=False)

    encT_d = nc.dram_tensor("encT", [BPC, H, T], f32r, kind="ExternalInput")
    wkT_d = nc.dram_tensor("wkT", [H, H], f32r, kind="ExternalInput")
    qb_d = nc.dram_tensor("qb", [128, HC, BPC], f32, kind="ExternalInput")
    wout_d = nc.dram_tensor("wout", [128, HC], f32r, kind="ExternalInput")
    out_d = nc.dram_tensor("out", [BPC, H], f32, kind="ExternalOutput")

    with tile.TileContext(nc) as tc:
        with (
            tc.tile_pool(name="weights", bufs=1) as wpool,
            tc.tile_pool(name="et", bufs=4) as et_pool,
            tc.tile_pool(name="tk", bufs=2) as tk_pool,
            tc.tile_pool(name="small", bufs=2) as small_pool,
            tc.tile_pool(name="prep", bufs=2) as prep_pool,
            tc.tile_pool(name="scratch", bufs=2) as scratch_pool,
            tc.tile_pool(name="parts", bufs=2) as parts_pool,
            tc.tile_pool(name="ktps", bufs=6, space="PSUM") as ktps_pool,
            tc.tile_pool(name="sps", bufs=2, space="PSUM") as sps_pool,
        ):
            # ---------------- persistent tiles ----------------
            wk_sb = wpool.tile([128, HC, H], f32r)    # Wk^T as [h_part, h_chunk, h']
            qb_sb = wpool.tile([128, HC, BPC], f32)   # tanh bias per (h' chunk, batch)
            wout_sb = wpool.tile([128, HC], f32r)
            ident = wpool.tile([128, 128], f32)
            z_sb = wpool.tile([1, BPC * NTILES + 3], f32)  # per-(tile|slice) partials

            masks.make_identity(nc, ident[:])

            # small inputs first, then tile-(0,0)/(0,1) streams interleaved
            # with the wk chunks so the PE can start ~10us in.
            pre_et0 = et_pool.tile([128, HC, TT], f32r, tag="et_sb")
            for i in range(HC):
                nc.sync.dma_start(
                    out=pre_et0[:, i, :],
                    in_=encT_d.ap()[0, i * 128 : (i + 1) * 128, 0:TT],
                )
                nc.sync.dma_start(
                    out=wk_sb[:, i, :],
                    in_=wkT_d.ap()[i * 128 : (i + 1) * 128, :],
                )
                if i == 0:
                    nc.sync.dma_start(out=qb_sb[:], in_=qb_d.ap())
                    nc.sync.dma_start(out=wout_sb[:], in_=wout_d.ap())
            pre_et1 = et_pool.tile([128, HC, TT], f32r, tag="et_sb")
            nc.sync.dma_start(
                out=pre_et1[:],
                in_=encT_d.ap()[0, :, TT : 2 * TT].rearrange("(i p) t -> p i t", p=128),
            )

            # ---------------- main loop ----------------
            for _rep in range(REPS):
                for b in range(BPC):
                    parts = parts_pool.tile([128, HC, NTILES + 3], f32)
                    for tt in range(NTILES):
                        first = _rep == 0 and b == 0
                        if first and tt == 0:
                            et_sb = pre_et0
                        elif first and tt == 1:
                            et_sb = pre_et1
                        else:
                            et_sb = et_pool.tile([128, HC, TT], f32r, tag="et_sb")
                            nc.sync.dma_start(
                                out=et_sb[:],
                                in_=encT_d.ap()[
                                    b, :, tt * TT : (tt + 1) * TT
                                ].rearrange("(i p) t -> p i t", p=128),
                            )

                        # K^T = Wk' x E^T ; tanh(K^T + qb) fused on ACT
                        tk_sb = tk_pool.tile([128, HC, TT], f32r)
                        for j in range(HC):
                            kt_ps = ktps_pool.tile([128, TT], f32, tag="ktps")
                            for i in range(HC):
                                nc.tensor.matmul(
                                    kt_ps[:],
                                    wk_sb[:, i, j * 128 : (j + 1) * 128],
                                    et_sb[:, i, :],
                                    start=(i == 0),
                                    stop=(i == HC - 1),
                                )
                            nc.scalar.activation(
                                tk_sb[:, j, :],
                                kt_ps[:],
                                AF.Tanh,
                                bias=qb_sb[:, j, b : b + 1],
                            )

                        # scores s[1, t] = sum_j Wout_j x tanhK^T_j, then
                        # p = exp(s) (accum_out -> Z partial), p broadcast on
                        # GpSimd, and the fused DVE mul+row-sum context.
                        # The very last tile runs this in two 256-col halves
                        # so the exp/broadcast/DVE chain of half 0 overlaps
                        # the scores of half 1, shrinking the exposed tail.
                        last = (
                            _rep == REPS - 1
                            and b == BPC - 1
                            and tt == NTILES - 1
                        )
                        halves = (
                            [(0, TT // 2, tt), (TT // 2, TT, NTILES)]
                            if last
                            else [(0, TT, tt)]
                        )
                        for lo, hi, zc in halves:
                            w = hi - lo
                            s_ps = sps_pool.tile([1, TT], f32, tag="sps")
                            for j in range(HC):
                                nc.tensor.matmul(
                                    s_ps[:, :w],
                                    wout_sb[:, j : j + 1],
                                    tk_sb[:, j, lo:hi],
                                    start=(j == 0),
                                    stop=(j == HC - 1),
                                )
                            p_sb = small_pool.tile([1, TT], f32, tag="p")
                            nc.scalar.activation(
                                p_sb[:, :w],
                                s_ps[:, :w],
                                AF.Exp,
                                accum_out=z_sb[
                                    :, b * NTILES + zc : b * NTILES + zc + 1
                                ],
                            )
                            p_rep = prep_pool.tile([128, TT], f32)
                            nc.gpsimd.partition_broadcast(
                                p_rep[:, :w], p_sb[:, :w]
                            )
                            for j in range(HC):
                                scr = scratch_pool.tile([128, TT], f32, tag="scr")
                                nc.vector.scalar_tensor_tensor(
                                    scr[:, :w],
                                    et_sb[:, j, lo:hi].bitcast(f32),
                                    1.0,
                                    p_rep[:, :w],
                                    op0=mybir.AluOpType.mult,
                                    op1=mybir.AluOpType.mult,
                                    accum_out=parts[:, j, zc : zc + 1],
                                )

                    # normalize: out[b] = ctx / Z, assembled via PE transpose
                    nz = NTILES + (
                        1 if (_rep == REPS - 1 and b == BPC - 1) else 0
                    )
                    ctxp = small_pool.tile([128, HC], f32, tag="ctxp")
                    nc.vector.tensor_reduce(
                        ctxp[:],
                        parts[:, :, 0:nz],
                        axis=mybir.AxisListType.X,
                        op=mybir.AluOpType.add,
                    )
                    zz = small_pool.tile([1, 1], f32, tag="zz")
                    nc.vector.tensor_reduce(
                        zz[:],
                        z_sb[:, b * NTILES : b * NTILES + nz],
                        axis=mybir.AxisListType.X,
                        op=mybir.AluOpType.add,
                    )
                    ct_ps = sps_pool.tile([HC, 128], f32, tag="sps")
                    nc.tensor.transpose(ct_ps[:], ctxp[:], ident[:])
                    rz = small_pool.tile([1, 1], f32, tag="rz")
                    nc.vector.reciprocal(rz[:], zz[:])
                    rz_rep = small_pool.tile([HC, 1], f32, tag="rzr")
                    nc.gpsimd.partition_broadcast(rz_rep[:], rz[:])
                    o_sb = small_pool.tile([HC, 128], f32, tag="o")
                    nc.vector.tensor_scalar_mul(o_sb[:], ct_ps[:], rz_rep[:])
                    nc.sync.dma_start(
                        out=out_d.ap()[b].rearrange("(i p) -> i p", p=128),
                        in_=o_sb[:],
                    )

    nc.compile()
    return nc


_NC_CACHE = None
_last_in_maps = None


def _host_prep(encoder_outputs, decoder_output, Wk, bk, Wq, bq, Wout, bout):
    enc = np.ascontiguousarray(np.asarray(encoder_outputs, dtype=np.float32))
    encT = np.ascontiguousarray(enc.transpose(0, 2, 1))         # [B, H, T]
    dec = np.asarray(decoder_output, dtype=np.float32)[0]       # [B, H]
    wkT = np.ascontiguousarray(np.asarray(Wk, dtype=np.float32).T)
    # qb[b, h'] = dec @ Wq.T + bq + bk  (tiny bias-vector precompute)
    qb = (
        dec.astype(np.float64) @ np.asarray(Wq, dtype=np.float64).T
        + np.asarray(bq, dtype=np.float64)
        + np.asarray(bk, dtype=np.float64)
    ).astype(np.float32)                                        # [B, H]
    wout_t = np.ascontiguousarray(
        np.asarray(Wout, dtype=np.float32).reshape(HC, 128).T
    )
    in_maps = []
    for c in range(NCORES):
        qb_c = qb[c * BPC : (c + 1) * BPC]                      # [BPC, H]
        # qb tile layout [p, j, b] = qb_c[b, j*128+p]
        qb_t = np.ascontiguousarray(
            np.transpose(qb_c.reshape(BPC, HC, 128), (2, 1, 0))
        )
        in_maps.append(
            {
                "encT": encT[c * BPC : (c + 1) * BPC],
                "wkT": wkT,
                "qb": qb_t,
                "wout": wout_t,
            }
        )
    return in_maps


def kernel(
    encoder_outputs,
    decoder_output,
    Wk,
    bk,
    Wq,
    bq,
    Wout,
    bout,
    inputs=None,
    **_unused,
):
    global _NC_CACHE, _last_in_maps
    from concourse.bass_utils import run_bass_kernel_spmd

    in_maps = _host_prep(
        encoder_outputs, decoder_output, Wk, bk, Wq, bq, Wout, bout
    )
    if _NC_CACHE is None:
        _NC_CACHE = _build()
    nc = _NC_CACHE
    _last_in_maps = in_maps
    res = run_bass_kernel_spmd(nc, in_maps, core_ids=list(range(NCORES)))
    ctx = np.concatenate([res.results[c]["out"] for c in range(NCORES)], axis=0)
    return ctx[:, None, :].astype(np.float32)                   # [B, 1, H]
